# revision 26
# baseline (speedup 1.0000x reference)
"""MultiHeadCrossAttention Trainium2 kernel.

Strategy: data-parallel over batch (8 batches -> 8 cores), no collectives.

Per-core computation (batch b):
  q^T = Wq^T-free form:  qT_out[e, r]  = sum_k Wq[k, e] * query[r, k]   (lhsT=Wq natural, rhs=query^T)
  k^T:  kT_out[e, l]     = sum_k Wk[k, e] * key[l, k]                    (lhsT=Wk natural, rhs=key^T)
  v:    v[l, f]          = sum_k value[l, k] * Wv[k, f]                  (lhsT=value^T blocks, rhs=Wv natural)
  scores^T[l, r] (head h) = sum_d kT[64h+d, l] * qT[64h+d, r]            (K=64 contraction)
  exp on ScalarE straight out of PSUM (scale=1/8 fused, no max-subtract; scores are small)
  ctx^T+totals via ones-column:  lhsT=[v_h|1] [128lk x 65], rhs=exp^T -> ctx_ext^T [65, 512]
  normalize: recip(totals) (DVE) -> partition_broadcast (GpSimd) -> multiply (DVE)
  out[r, e] = sum_dv ctx^T[dv, r] * Wo[dv, e] + bo                       (lhsT=ctx^T, rhs=Wo natural)

All matmul operands bf16 (host-cast); PSUM accumulation fp32; biases fp32; output fp32.
Host pre-transposes query/key/value and pre-tiles Wq/Wk/value so every DMA is contiguous.
"""

import sys

if "/opt/trn_rl_repo" not in sys.path:
    sys.path.insert(0, "/opt/trn_rl_repo")

import numpy as np
import ml_dtypes

import concourse.bass as bass
import concourse.tile as tile
from concourse import bacc
from concourse import mybir
from concourse.bass_utils import run_bass_kernel_spmd

BF16 = ml_dtypes.bfloat16

B, LQ, LK = 8, 512, 1024
EMB, VIN, VOUT = 1024, 512, 1024
H = 16
DH = EMB // H   # 64
DV = VOUT // H  # 64
P = 128
SCALE = 1.0 / 8.0  # 1/sqrt(DH)

EC = EMB // P    # 8 e-chunks
KC = EMB // P    # 8 k-chunks (emb contraction)
VC = VIN // P    # 4 vin-chunks
JC = LK // P     # 8 lk-chunks
MC = LQ // P     # 4 row-chunks
NQ = LQ          # moving free dim for scores/ctx (=512)

F32 = mybir.dt.float32
BF = mybir.dt.bfloat16


def build_module():
    nc = bacc.Bacc("TRN2", target_bir_lowering=False)

    # ---- DRAM parameters (per-core layouts, host pre-arranged) ----
    qT = nc.declare_dram_parameter("qT", [EMB, LQ], BF, isOutput=False)          # query[b].T
    keyT = nc.declare_dram_parameter("keyT", [VIN, LK], BF, isOutput=False)      # key[b].T
    # value[b].T tiled: [j=8][p=128 vin-in-chunk ... ] -> [JC, P, VC, P] (see host prep)
    valT = nc.declare_dram_parameter("valT", [JC, P, VC, P], BF, isOutput=False)
    # Wq column-tiles, SBUF-ready: wq[m, p, c, e] = Wq[c*128+p, m*128+e]
    wq = nc.declare_dram_parameter("wq", [EC, P, KC, P], BF, isOutput=False)
    wk = nc.declare_dram_parameter("wk", [EC, P, VC, P], BF, isOutput=False)
    wv = nc.declare_dram_parameter("wv", [VIN, VOUT], BF, isOutput=False)        # natural
    wo = nc.declare_dram_parameter("wo", [VOUT, EMB], BF, isOutput=False)        # natural
    bq = nc.declare_dram_parameter("bq", [P, EC], F32, isOutput=False)
    bk = nc.declare_dram_parameter("bk", [P, EC], F32, isOutput=False)
    bv = nc.declare_dram_parameter("bv", [VOUT], F32, isOutput=False)
    bo = nc.declare_dram_parameter("bo", [EMB], F32, isOutput=False)
    out = nc.declare_dram_parameter("out", [LQ, EMB], F32, isOutput=True)

    with tile.TileContext(nc) as tc:
        with (
            tc.tile_pool(name="res", bufs=1) as res,          # resident tiles
            tc.tile_pool(name="wstream", bufs=3) as wstream,  # streamed weight tiles
            tc.tile_pool(name="acts", bufs=1) as acts,        # produced activations (resident)
            tc.tile_pool(name="expp", bufs=36) as expp,       # exp score tiles
            tc.tile_pool(name="smalls", bufs=2) as smalls,    # recip / bcast tiles
            tc.tile_pool(name="outp", bufs=2) as outp,
            tc.tile_pool(name="stage", bufs=6) as stage,
            tc.tile_pool(name="dramp", bufs=4, space="DRAM") as dramp,        # output staging
            tc.tile_pool(name="psum2", bufs=3, space="PSUM") as psum2,  # 2-bank tiles
            tc.tile_pool(name="psum1", bufs=2, space="PSUM") as psum1,  # 1-bank tiles
        ):
            # ---- resident + stream DMAs (per-queue first-need order) ----
            # SP queue: wk0, keyT, bk, wq0, vt0, then later pairs' streams
            wk_t = [wstream.tile([P, VC, P], BF, tag="wk_m", name=f"wk_m{p}") for p in range(EC)]
            wq_t = [wstream.tile([P, KC, P], BF, tag="wq_m", name=f"wq_m{p}") for p in range(EC)]
            vt_t = [wstream.tile([P, VC, P], BF, tag="vt", name=f"vt{p}") for p in range(EC)]
            nc.sync.dma_start(out=wk_t[0], in_=wk[0])
            keyT_in = []
            for c in range(VC):
                t = res.tile([P, LK], BF, tag=f"keyT_{c}", name=f"keyT_{c}")
                nc.sync.dma_start(out=t, in_=keyT[c * P:(c + 1) * P, :])
                keyT_in.append(t)
            bk_sb = res.tile([P, EC], F32, tag="bk")
            nc.sync.dma_start(out=bk_sb, in_=bk[:, :])
            nc.sync.dma_start(out=wq_t[0], in_=wq[0])
            nc.sync.dma_start(out=vt_t[0], in_=valT[0])
            # qT split across SP + POOL queues for parallel load
            qT_in = []
            for c in range(KC):
                t = res.tile([P, LQ], BF, tag=f"qT_{c}", name=f"qT_{c}")
                eng_q = nc.sync if c < 4 else nc.gpsimd
                eng_q.dma_start(out=t, in_=qT[c * P:(c + 1) * P, :])
                qT_in.append(t)
            bq_sb = res.tile([P, EC], F32, tag="bq")
            nc.gpsimd.dma_start(out=bq_sb, in_=bq[:, :])
            wv_in = []
            for c in range(VC):
                t = res.tile([P, VOUT], BF, tag=f"wv_{c}", name=f"wv_{c}")
                nc.gpsimd.dma_start(out=t, in_=wv[c * P:(c + 1) * P, :])
                wv_in.append(t)
            bv_bc = res.tile([P, VOUT], F32, tag="bv")
            nc.gpsimd.dma_start(out=bv_bc, in_=bv[None, :].to_broadcast((P, VOUT)))

            # ---- produced activations (resident SBUF) ----
            qT_sb = [acts.tile([P, LQ], BF, tag=f"qTs_{m}", name=f"qTs_{m}") for m in range(EC)]
            kT_sb = [acts.tile([P, LK], BF, tag=f"kTs_{m}", name=f"kTs_{m}") for m in range(EC)]
            v_ext = [acts.tile([P, H, DV + 1], BF, tag=f"vext_{j}", name=f"vext_{j}") for j in range(JC)]
            ctxT_sb = [acts.tile([P, LQ], BF, tag=f"ctxT_{c}", name=f"ctxT_{c}") for c in range(VOUT // P)]

            exp_sb = {}  # (h, t) -> tile, filled during attention

            def emit_scores(h):
                off = DH * (h % 2)
                pair = h // 2
                for t in range(4):
                    sc = psum2.tile([P, 2, NQ], F32, tag="p2", name=f"sc_{h}_{t}")
                    for i in range(2):
                        j = 2 * t + i
                        nc.tensor.matmul(
                            sc[:, i, :],
                            lhsT=kT_sb[pair][off:off + DH, j * P:(j + 1) * P],
                            rhs=qT_sb[pair][off:off + DH, :],
                            start=True,
                            stop=True,
                        )
                    e = expp.tile([P, 2, NQ], BF, tag="exp", name=f"exp_{h}_{t}")
                    nc.scalar.activation(
                        out=e.rearrange("p a b -> p (a b)"),
                        in_=sc.rearrange("p a b -> p (a b)"),
                        func=mybir.ActivationFunctionType.Exp,
                        scale=SCALE,
                    )
                    exp_sb[(h, t)] = e

            tot_dram = dramp.tile([H, NQ], F32, tag="totd", name="tot_dram")
            stage_sb = {}

            def emit_ctx(h):
                off = DH * (h % 2)
                pair = h // 2
                cx = psum1.tile([DV + 1, NQ], F32, tag="p1", name=f"cx_{h}")
                for t in range(4):
                    e = exp_sb[(h, t)]
                    for i in range(2):
                        j = 2 * t + i
                        nc.tensor.matmul(
                            cx,
                            lhsT=v_ext[j][:, h, :],
                            rhs=e[:, i, :],
                            start=(j == 0),
                            stop=(j == JC - 1),
                        )
                # ctx_ext (incl. totals row) -> SBUF staging; frees PSUM fast
                st = stage.tile([DV + 1, NQ], F32, tag="stg", name=f"st_{h}")
                nc.vector.tensor_copy(out=st, in_=cx[:])
                nc.sync.dma_start(out=tot_dram[h:h + 1, :], in_=st[DV:DV + 1, :])
                stage_sb[h] = st

            def emit_norm_group(g):
                h0 = 4 * g
                tg = smalls.tile([4, NQ], F32, tag="totg", name=f"tg_{g}")
                nc.sync.dma_start(out=tg, in_=tot_dram[h0:h0 + 4, :])
                rec = smalls.tile([4, NQ], F32, tag="recg", name=f"rec_{g}")
                nc.vector.reciprocal(out=rec, in_=tg)
                recd = dramp.tile([4, NQ], F32, tag="recd", name=f"recd_{g}")
                nc.sync.dma_start(out=recd, in_=rec)
                for hh in range(h0, h0 + 4):
                    off = DH * (hh % 2)
                    pair = hh // 2
                    bc = smalls.tile([DV, NQ], F32, tag="bcast", name=f"bc_{hh}")
                    nc.sync.dma_start(
                        out=bc, in_=recd[hh - h0:hh - h0 + 1, :].to_broadcast((DV, NQ)))
                    nc.vector.tensor_tensor(
                        out=ctxT_sb[pair][off:off + DV, :],
                        in0=stage_sb[hh][0:DV, :],
                        in1=bc[:],
                        op=mybir.AluOpType.mult,
                    )

            # =========================================================
            # Phase 1 (per pair p): K-proj(p), Q-proj(p), scores(early),
            # V-proj(j=p)
            # =========================================================
            EARLY = {0: [0, 1], 1: [2, 3], 2: [4, 5], 3: [6, 7], 4: [8]}
            for p in range(EC):
                if p > 0:
                    nc.sync.dma_start(out=wk_t[p], in_=wk[p])
                    nc.sync.dma_start(out=wq_t[p], in_=wq[p])
                    nc.sync.dma_start(out=vt_t[p], in_=valT[p])
                # ---- K projection, e-chunk p ----
                wk_m = wk_t[p]
                kq = psum2.tile([P, 2, NQ], F32, tag="p2")
                for n in range(2):
                    for c in range(VC):
                        nc.tensor.matmul(
                            kq[:, n, :],
                            lhsT=wk_m[:, c, :],
                            rhs=keyT_in[c][:, n * NQ:(n + 1) * NQ],
                            start=(c == 0),
                            stop=(c == VC - 1),
                        )
                nc.vector.tensor_tensor(
                    out=kT_sb[p][:], in0=kq.rearrange("p a b -> p (a b)"),
                    in1=bk_sb[:, p:p + 1].to_broadcast((P, LK)),
                    op=mybir.AluOpType.add,
                )

                # ---- Q projection, e-chunk p ----
                wq_m = wq_t[p]
                qp = psum1.tile([P, NQ], F32, tag="p1")
                for c in range(KC):
                    nc.tensor.matmul(
                        qp,
                        lhsT=wq_m[:, c, :],
                        rhs=qT_in[c][:],
                        start=(c == 0),
                        stop=(c == KC - 1),
                    )
                nc.vector.tensor_tensor(
                    out=qT_sb[p][:], in0=qp[:],
                    in1=bq_sb[:, p:p + 1].to_broadcast((P, LQ)),
                    op=mybir.AluOpType.add,
                )

                # ---- early scores (feed ACT ASAP), before V-proj ----
                for eh in EARLY.get(p, []):
                    emit_scores(eh)

                # ---- V projection, lk-chunk j=p ----
                vt = vt_t[p]
                vp = psum2.tile([P, 2, NQ], F32, tag="p2")
                for n in range(2):
                    for c in range(VC):
                        nc.tensor.matmul(
                            vp[:, n, :],
                            lhsT=vt[:, c, :],
                            rhs=wv_in[c][:, n * NQ:(n + 1) * NQ],
                            start=(c == 0),
                            stop=(c == VC - 1),
                        )
                for n in range(2):
                    nc.vector.tensor_tensor(
                        out=v_ext[p][:, n * 8:(n + 1) * 8, 0:DV],
                        in0=vp[:, n, :].rearrange("p (a b) -> p a b", a=8),
                        in1=bv_bc[:, n * NQ:(n + 1) * NQ].rearrange("p (a b) -> p a b", a=8),
                        op=mybir.AluOpType.add,
                    )
                nc.vector.memset(v_ext[p][:, :, DV:DV + 1], 1.0)

            # =========================================================
            # Phase 2 per head: scores -> exp (ACT) -> ctx, with
            # lookahead-1 on scores so ACT stays ahead of PE.
            # =========================================================
            wo_in = []
            for c in range(VOUT // P):
                t = res.tile([P, EMB], BF, tag=f"qT_{c}", name=f"wo_{c}")
                nc.gpsimd.dma_start(out=t, in_=wo[c * P:(c + 1) * P, :])
                wo_in.append(t)
            bo_bc = res.tile([P, EMB], F32, tag="bo")
            nc.gpsimd.dma_start(out=bo_bc, in_=bo[None, :].to_broadcast((P, EMB)))

            LOOK = 8
            for h in range(H):
                if h + LOOK < H and h + LOOK > 8:
                    emit_scores(h + LOOK)
                emit_ctx(h)
                if h % 4 == 3:
                    emit_norm_group(h // 4)

            # =========================================================
            # Phase 3: output projection
            # =========================================================
            for m0 in (0, 2):
                pos = [psum2.tile([P, 2, NQ], F32, tag="p2", name=f"po_{m0}_{k}")
                       for k in range(2)]
                for c in range(VOUT // P):
                    for k in range(2):
                        m = m0 + k
                        for n in range(2):
                            nc.tensor.matmul(
                                pos[k][:, n, :],
                                lhsT=ctxT_sb[c][:, m * P:(m + 1) * P],
                                rhs=wo_in[c][:, n * NQ:(n + 1) * NQ],
                                start=(c == 0),
                                stop=(c == VOUT // P - 1),
                            )
                for k in range(2):
                    m = m0 + k
                    ot = outp.tile([P, EMB], F32, tag="out", name=f"ot_{m}")
                    nc.vector.tensor_tensor(
                        out=ot,
                        in0=pos[k].rearrange("p a b -> p (a b)"),
                        in1=bo_bc[:],
                        op=mybir.AluOpType.add,
                    )
                    nc.sync.dma_start(out=out[m * P:(m + 1) * P, :], in_=ot)

    nc.compile()
    return nc


_NC_CACHE = {}


def _get_module():
    if "nc" not in _NC_CACHE:
        _NC_CACHE["nc"] = build_module()
    return _NC_CACHE["nc"]


def _prep_core_inputs(query, key, value, wq_t, wk_t, wv_b, wo_b, bq, bk, bv, bo, b):
    qT = np.ascontiguousarray(query[b].T).astype(BF16)              # [1024, 512]
    keyT = np.ascontiguousarray(key[b].T).astype(BF16)              # [512, 1024]
    # value[b].T [512, 1024] -> [j, p(vin%128), c(vin//128), e] tiles
    vT = value[b].T.reshape(VC, P, JC, P).transpose(2, 1, 0, 3)     # [8, 128, 4, 128]
    valT = np.ascontiguousarray(vT).astype(BF16)
    return {
        "qT": qT, "keyT": keyT, "valT": valT,
        "wq": wq_t, "wk": wk_t, "wv": wv_b, "wo": wo_b,
        "bq": bq, "bk": bk, "bv": bv, "bo": bo,
    }


def kernel(query, key, value, Wq, bq, Wk, bk, Wv, bv, Wo, bo, _want_profile=False):
    nc = _get_module()

    # weight prep (shared across cores)
    # wq[m, p, c, e] = Wq[c*128+p, m*128+e]
    wq_t = np.ascontiguousarray(
        Wq.reshape(KC, P, EC, P).transpose(2, 1, 0, 3)).astype(BF16)  # [8, 128, 8, 128]
    wk_t = np.ascontiguousarray(
        Wk.reshape(VC, P, EC, P).transpose(2, 1, 0, 3)).astype(BF16)  # [8, 128, 4, 128]
    wv_b = Wv.astype(BF16)
    wo_b = Wo.astype(BF16)
    bq32 = np.ascontiguousarray(np.asarray(bq, np.float32).reshape(EC, P).T)  # [128, 8]
    bk32 = np.ascontiguousarray(np.asarray(bk, np.float32).reshape(EC, P).T)  # [128, 8]
    bv32 = np.asarray(bv, np.float32)
    bo32 = np.asarray(bo, np.float32)

    in_maps = [
        _prep_core_inputs(query, key, value, wq_t, wk_t, wv_b, wo_b,
                          bq32, bk32, bv32, bo32, b)
        for b in range(B)
    ]
    res = run_bass_kernel_spmd(nc, in_maps, core_ids=list(range(B)),
                               trace=_want_profile)
    out = np.stack([res.results[c]["out"] for c in range(B)], axis=0)
    if _want_profile:
        return out, res
    return out


# revision 27
# speedup vs baseline: 1.0804x; 1.0804x over previous
"""MultiHeadCrossAttention Trainium2 kernel.

Strategy: data-parallel over batch (8 batches -> 8 cores), no collectives.

Per-core computation (batch b):
  q^T = Wq^T-free form:  qT_out[e, r]  = sum_k Wq[k, e] * query[r, k]   (lhsT=Wq natural, rhs=query^T)
  k^T:  kT_out[e, l]     = sum_k Wk[k, e] * key[l, k]                    (lhsT=Wk natural, rhs=key^T)
  v:    v[l, f]          = sum_k value[l, k] * Wv[k, f]                  (lhsT=value^T blocks, rhs=Wv natural)
  scores^T[l, r] (head h) = sum_d kT[64h+d, l] * qT[64h+d, r]            (K=64 contraction)
  exp on ScalarE straight out of PSUM (scale=1/8 fused, no max-subtract; scores are small)
  ctx^T+totals via ones-column:  lhsT=[v_h|1] [128lk x 65], rhs=exp^T -> ctx_ext^T [65, 512]
  normalize: recip(totals) (DVE) -> partition_broadcast (GpSimd) -> multiply (DVE)
  out[r, e] = sum_dv ctx^T[dv, r] * Wo[dv, e] + bo                       (lhsT=ctx^T, rhs=Wo natural)

All matmul operands bf16 (host-cast); PSUM accumulation fp32; biases fp32; output fp32.
Host pre-transposes query/key/value and pre-tiles Wq/Wk/value so every DMA is contiguous.
"""

import sys

if "/opt/trn_rl_repo" not in sys.path:
    sys.path.insert(0, "/opt/trn_rl_repo")

import numpy as np
import ml_dtypes

import concourse.bass as bass
import concourse.tile as tile
from concourse import bacc
from concourse import mybir
from concourse.bass_utils import run_bass_kernel_spmd

BF16 = ml_dtypes.bfloat16

B, LQ, LK = 8, 512, 1024
EMB, VIN, VOUT = 1024, 512, 1024
H = 16
DH = EMB // H   # 64
DV = VOUT // H  # 64
P = 128
SCALE = 1.0 / 8.0  # 1/sqrt(DH)

EC = EMB // P    # 8 e-chunks
KC = EMB // P    # 8 k-chunks (emb contraction)
VC = VIN // P    # 4 vin-chunks
JC = LK // P     # 8 lk-chunks
MC = LQ // P     # 4 row-chunks
NQ = LQ          # moving free dim for scores/ctx (=512)

F32 = mybir.dt.float32
BF = mybir.dt.bfloat16


def build_module():
    nc = bacc.Bacc("TRN2", target_bir_lowering=False)

    # ---- DRAM parameters (per-core layouts, host pre-arranged) ----
    qT = nc.declare_dram_parameter("qT", [EMB, LQ], BF, isOutput=False)          # query[b].T
    keyT = nc.declare_dram_parameter("keyT", [VIN, LK], BF, isOutput=False)      # key[b].T
    # value[b].T tiled: [j=8][p=128 vin-in-chunk ... ] -> [JC, P, VC, P] (see host prep)
    valT = nc.declare_dram_parameter("valT", [JC, P, VC, P], BF, isOutput=False)
    # Wq column-tiles, SBUF-ready: wq[m, p, c, e] = Wq[c*128+p, m*128+e]
    wq = nc.declare_dram_parameter("wq", [EC, P, KC, P], BF, isOutput=False)
    wk = nc.declare_dram_parameter("wk", [EC, P, VC, P], BF, isOutput=False)
    wv = nc.declare_dram_parameter("wv", [VIN, VOUT], BF, isOutput=False)        # natural
    wo = nc.declare_dram_parameter("wo", [VOUT, EMB], BF, isOutput=False)        # natural
    bq = nc.declare_dram_parameter("bq", [P, EC], F32, isOutput=False)
    bk = nc.declare_dram_parameter("bk", [P, EC], F32, isOutput=False)
    bv = nc.declare_dram_parameter("bv", [VOUT], F32, isOutput=False)
    bo = nc.declare_dram_parameter("bo", [EMB], F32, isOutput=False)
    out = nc.declare_dram_parameter("out", [LQ, EMB], F32, isOutput=True)

    with tile.TileContext(nc) as tc:
        with (
            tc.tile_pool(name="res", bufs=1) as res,          # resident tiles
            tc.tile_pool(name="wstream", bufs=3) as wstream,  # streamed weight tiles
            tc.tile_pool(name="acts", bufs=1) as acts,        # produced activations (resident)
            tc.tile_pool(name="expp", bufs=36) as expp,       # exp score tiles
            tc.tile_pool(name="smalls", bufs=2) as smalls,    # recip / bcast tiles
            tc.tile_pool(name="outp", bufs=2) as outp,
            tc.tile_pool(name="stage", bufs=6) as stage,
            tc.tile_pool(name="dramp", bufs=4, space="DRAM") as dramp,        # output staging
            tc.tile_pool(name="psum2", bufs=3, space="PSUM") as psum2,  # 2-bank tiles
            tc.tile_pool(name="psum1", bufs=2, space="PSUM") as psum1,  # 1-bank tiles
        ):
            # ---- resident + stream DMAs (per-queue first-need order) ----
            # SP queue: wk0, keyT, bk, wq0, vt0, then later pairs' streams
            wk_t = [wstream.tile([P, VC, P], BF, tag="wk_m", name=f"wk_m{p}") for p in range(EC)]
            wq_t = [wstream.tile([P, KC, P], BF, tag="wq_m", name=f"wq_m{p}") for p in range(EC)]
            vt_t = [wstream.tile([P, VC, P], BF, tag="vt", name=f"vt{p}") for p in range(EC)]
            nc.sync.dma_start(out=wk_t[0], in_=wk[0])
            keyT_in = []
            for c in range(VC):
                t = res.tile([P, LK], BF, tag=f"keyT_{c}", name=f"keyT_{c}")
                nc.sync.dma_start(out=t, in_=keyT[c * P:(c + 1) * P, :])
                keyT_in.append(t)
            bk_sb = res.tile([P, EC], F32, tag="bk")
            nc.sync.dma_start(out=bk_sb, in_=bk[:, :])
            nc.sync.dma_start(out=wq_t[0], in_=wq[0])
            nc.sync.dma_start(out=vt_t[0], in_=valT[0])
            # qT split across SP + POOL queues for parallel load
            qT_in = []
            for c in range(KC):
                t = res.tile([P, LQ], BF, tag=f"qT_{c}", name=f"qT_{c}")
                eng_q = nc.sync if c < 4 else nc.gpsimd
                eng_q.dma_start(out=t, in_=qT[c * P:(c + 1) * P, :])
                qT_in.append(t)
            bq_sb = res.tile([P, EC], F32, tag="bq")
            nc.gpsimd.dma_start(out=bq_sb, in_=bq[:, :])
            wv_in = []
            for c in range(VC):
                t = res.tile([P, VOUT], BF, tag=f"wv_{c}", name=f"wv_{c}")
                nc.gpsimd.dma_start(out=t, in_=wv[c * P:(c + 1) * P, :])
                wv_in.append(t)
            bv_bc = res.tile([P, VOUT], F32, tag="bv")
            nc.gpsimd.dma_start(out=bv_bc, in_=bv[None, :].to_broadcast((P, VOUT)))

            # ---- produced activations (resident SBUF) ----
            qT_sb = [acts.tile([P, LQ], BF, tag=f"qTs_{m}", name=f"qTs_{m}") for m in range(EC)]
            kT_sb = [acts.tile([P, LK], BF, tag=f"kTs_{m}", name=f"kTs_{m}") for m in range(EC)]
            v_ext = [acts.tile([P, H, DV + 1], BF, tag=f"vext_{j}", name=f"vext_{j}") for j in range(JC)]
            ctxT_sb = [acts.tile([P, LQ], BF, tag=f"ctxT_{c}", name=f"ctxT_{c}") for c in range(VOUT // P)]

            exp_sb = {}  # (h, t) -> tile, filled during attention

            def emit_scores(h):
                off = DH * (h % 2)
                pair = h // 2
                for t in range(4):
                    sc = psum2.tile([P, 2, NQ], F32, tag="p2", name=f"sc_{h}_{t}")
                    for i in range(2):
                        j = 2 * t + i
                        nc.tensor.matmul(
                            sc[:, i, :],
                            lhsT=kT_sb[pair][off:off + DH, j * P:(j + 1) * P],
                            rhs=qT_sb[pair][off:off + DH, :],
                            start=True,
                            stop=True,
                        )
                    e = expp.tile([P, 2, NQ], BF, tag="exp", name=f"exp_{h}_{t}")
                    nc.scalar.activation(
                        out=e.rearrange("p a b -> p (a b)"),
                        in_=sc.rearrange("p a b -> p (a b)"),
                        func=mybir.ActivationFunctionType.Exp,
                        scale=SCALE,
                    )
                    exp_sb[(h, t)] = e

            tot_dram = dramp.tile([H, NQ], F32, tag="totd", name="tot_dram")
            stage_sb = {}

            def emit_ctx(h):
                off = DH * (h % 2)
                pair = h // 2
                cx = psum1.tile([DV + 1, NQ], F32, tag="p1", name=f"cx_{h}")
                for t in range(4):
                    e = exp_sb[(h, t)]
                    for i in range(2):
                        j = 2 * t + i
                        nc.tensor.matmul(
                            cx,
                            lhsT=v_ext[j][:, h, :],
                            rhs=e[:, i, :],
                            start=(j == 0),
                            stop=(j == JC - 1),
                        )
                # ctx_ext (incl. totals row) -> SBUF staging; frees PSUM fast
                st = stage.tile([DV + 1, NQ], F32, tag="stg", name=f"st_{h}")
                nc.vector.tensor_copy(out=st, in_=cx[:])
                nc.sync.dma_start(out=tot_dram[h:h + 1, :], in_=st[DV:DV + 1, :])
                stage_sb[h] = st

            def emit_norm_group(g):
                h0 = 4 * g
                tg = smalls.tile([4, NQ], F32, tag="totg", name=f"tg_{g}")
                nc.sync.dma_start(out=tg, in_=tot_dram[h0:h0 + 4, :])
                rec = smalls.tile([4, NQ], F32, tag="recg", name=f"rec_{g}")
                nc.vector.reciprocal(out=rec, in_=tg)
                recd = dramp.tile([4, NQ], F32, tag="recd", name=f"recd_{g}")
                nc.sync.dma_start(out=recd, in_=rec)
                for hh in range(h0, h0 + 4):
                    off = DH * (hh % 2)
                    pair = hh // 2
                    bc = smalls.tile([DV, NQ], F32, tag="bcast", name=f"bc_{hh}")
                    nc.sync.dma_start(
                        out=bc, in_=recd[hh - h0:hh - h0 + 1, :].to_broadcast((DV, NQ)))
                    nc.vector.tensor_tensor(
                        out=ctxT_sb[pair][off:off + DV, :],
                        in0=stage_sb[hh][0:DV, :],
                        in1=bc[:],
                        op=mybir.AluOpType.mult,
                    )

            # =========================================================
            # Phase 1 (per pair p): K-proj(p), Q-proj(p), scores(early),
            # V-proj(j=p)
            # =========================================================
            EARLY = {0: [0, 1], 1: [2, 3], 2: [4], 3: [5], 4: [6], 5: [7], 6: [8]}
            for p in range(EC):
                if p > 0:
                    nc.sync.dma_start(out=wk_t[p], in_=wk[p])
                    nc.sync.dma_start(out=wq_t[p], in_=wq[p])
                    nc.sync.dma_start(out=vt_t[p], in_=valT[p])
                # ---- K projection, e-chunk p ----
                wk_m = wk_t[p]
                kq = psum2.tile([P, 2, NQ], F32, tag="p2")
                for n in range(2):
                    for c in range(VC):
                        nc.tensor.matmul(
                            kq[:, n, :],
                            lhsT=wk_m[:, c, :],
                            rhs=keyT_in[c][:, n * NQ:(n + 1) * NQ],
                            start=(c == 0),
                            stop=(c == VC - 1),
                        )
                nc.vector.tensor_tensor(
                    out=kT_sb[p][:], in0=kq.rearrange("p a b -> p (a b)"),
                    in1=bk_sb[:, p:p + 1].to_broadcast((P, LK)),
                    op=mybir.AluOpType.add,
                )

                # ---- Q projection, e-chunk p ----
                wq_m = wq_t[p]
                qp = psum1.tile([P, NQ], F32, tag="p1")
                for c in range(KC):
                    nc.tensor.matmul(
                        qp,
                        lhsT=wq_m[:, c, :],
                        rhs=qT_in[c][:],
                        start=(c == 0),
                        stop=(c == KC - 1),
                    )
                nc.vector.tensor_tensor(
                    out=qT_sb[p][:], in0=qp[:],
                    in1=bq_sb[:, p:p + 1].to_broadcast((P, LQ)),
                    op=mybir.AluOpType.add,
                )

                # ---- early scores (feed ACT ASAP), before V-proj ----
                for eh in EARLY.get(p, []):
                    emit_scores(eh)

                # ---- V projection, lk-chunk j=p ----
                vt = vt_t[p]
                vp = psum2.tile([P, 2, NQ], F32, tag="p2")
                for n in range(2):
                    for c in range(VC):
                        nc.tensor.matmul(
                            vp[:, n, :],
                            lhsT=vt[:, c, :],
                            rhs=wv_in[c][:, n * NQ:(n + 1) * NQ],
                            start=(c == 0),
                            stop=(c == VC - 1),
                        )
                for n in range(2):
                    nc.vector.tensor_tensor(
                        out=v_ext[p][:, n * 8:(n + 1) * 8, 0:DV],
                        in0=vp[:, n, :].rearrange("p (a b) -> p a b", a=8),
                        in1=bv_bc[:, n * NQ:(n + 1) * NQ].rearrange("p (a b) -> p a b", a=8),
                        op=mybir.AluOpType.add,
                    )
                nc.vector.memset(v_ext[p][:, :, DV:DV + 1], 1.0)

            # =========================================================
            # Phase 2 per head: scores -> exp (ACT) -> ctx, with
            # lookahead-1 on scores so ACT stays ahead of PE.
            # =========================================================
            wo_in = []
            for c in range(VOUT // P):
                t = res.tile([P, EMB], BF, tag=f"qT_{c}", name=f"wo_{c}")
                nc.gpsimd.dma_start(out=t, in_=wo[c * P:(c + 1) * P, :])
                wo_in.append(t)
            bo_bc = res.tile([P, EMB], F32, tag="bo")
            nc.gpsimd.dma_start(out=bo_bc, in_=bo[None, :].to_broadcast((P, EMB)))

            LOOK = 8
            for h in range(H):
                if h + LOOK < H and h + LOOK > 8:
                    emit_scores(h + LOOK)
                emit_ctx(h)
                if h % 4 == 3:
                    emit_norm_group(h // 4)

            # =========================================================
            # Phase 3: output projection
            # =========================================================
            for m0 in (0, 2):
                pos = [psum2.tile([P, 2, NQ], F32, tag="p2", name=f"po_{m0}_{k}")
                       for k in range(2)]
                for c in range(VOUT // P):
                    for k in range(2):
                        m = m0 + k
                        for n in range(2):
                            nc.tensor.matmul(
                                pos[k][:, n, :],
                                lhsT=ctxT_sb[c][:, m * P:(m + 1) * P],
                                rhs=wo_in[c][:, n * NQ:(n + 1) * NQ],
                                start=(c == 0),
                                stop=(c == VOUT // P - 1),
                            )
                for k in range(2):
                    m = m0 + k
                    ot = outp.tile([P, EMB], F32, tag="out", name=f"ot_{m}")
                    nc.vector.tensor_tensor(
                        out=ot,
                        in0=pos[k].rearrange("p a b -> p (a b)"),
                        in1=bo_bc[:],
                        op=mybir.AluOpType.add,
                    )
                    nc.sync.dma_start(out=out[m * P:(m + 1) * P, :], in_=ot)

    nc.compile()
    return nc


_NC_CACHE = {}


def _get_module():
    if "nc" not in _NC_CACHE:
        _NC_CACHE["nc"] = build_module()
    return _NC_CACHE["nc"]


def _prep_core_inputs(query, key, value, wq_t, wk_t, wv_b, wo_b, bq, bk, bv, bo, b):
    qT = np.ascontiguousarray(query[b].T).astype(BF16)              # [1024, 512]
    keyT = np.ascontiguousarray(key[b].T).astype(BF16)              # [512, 1024]
    # value[b].T [512, 1024] -> [j, p(vin%128), c(vin//128), e] tiles
    vT = value[b].T.reshape(VC, P, JC, P).transpose(2, 1, 0, 3)     # [8, 128, 4, 128]
    valT = np.ascontiguousarray(vT).astype(BF16)
    return {
        "qT": qT, "keyT": keyT, "valT": valT,
        "wq": wq_t, "wk": wk_t, "wv": wv_b, "wo": wo_b,
        "bq": bq, "bk": bk, "bv": bv, "bo": bo,
    }


def kernel(query, key, value, Wq, bq, Wk, bk, Wv, bv, Wo, bo, _want_profile=False):
    nc = _get_module()

    # weight prep (shared across cores)
    # wq[m, p, c, e] = Wq[c*128+p, m*128+e]
    wq_t = np.ascontiguousarray(
        Wq.reshape(KC, P, EC, P).transpose(2, 1, 0, 3)).astype(BF16)  # [8, 128, 8, 128]
    wk_t = np.ascontiguousarray(
        Wk.reshape(VC, P, EC, P).transpose(2, 1, 0, 3)).astype(BF16)  # [8, 128, 4, 128]
    wv_b = Wv.astype(BF16)
    wo_b = Wo.astype(BF16)
    bq32 = np.ascontiguousarray(np.asarray(bq, np.float32).reshape(EC, P).T)  # [128, 8]
    bk32 = np.ascontiguousarray(np.asarray(bk, np.float32).reshape(EC, P).T)  # [128, 8]
    bv32 = np.asarray(bv, np.float32)
    bo32 = np.asarray(bo, np.float32)

    in_maps = [
        _prep_core_inputs(query, key, value, wq_t, wk_t, wv_b, wo_b,
                          bq32, bk32, bv32, bo32, b)
        for b in range(B)
    ]
    res = run_bass_kernel_spmd(nc, in_maps, core_ids=list(range(B)),
                               trace=_want_profile)
    out = np.stack([res.results[c]["out"] for c in range(B)], axis=0)
    if _want_profile:
        return out, res
    return out


# revision 28
# speedup vs baseline: 1.1297x; 1.0456x over previous
"""MultiHeadCrossAttention Trainium2 kernel.

Strategy: data-parallel over batch (8 batches -> 8 cores), no collectives.

Per-core computation (batch b):
  q^T = Wq^T-free form:  qT_out[e, r]  = sum_k Wq[k, e] * query[r, k]   (lhsT=Wq natural, rhs=query^T)
  k^T:  kT_out[e, l]     = sum_k Wk[k, e] * key[l, k]                    (lhsT=Wk natural, rhs=key^T)
  v:    v[l, f]          = sum_k value[l, k] * Wv[k, f]                  (lhsT=value^T blocks, rhs=Wv natural)
  scores^T[l, r] (head h) = sum_d kT[64h+d, l] * qT[64h+d, r]            (K=64 contraction)
  exp on ScalarE straight out of PSUM (scale=1/8 fused, no max-subtract; scores are small)
  ctx^T+totals via ones-column:  lhsT=[v_h|1] [128lk x 65], rhs=exp^T -> ctx_ext^T [65, 512]
  normalize: recip(totals) (DVE) -> partition_broadcast (GpSimd) -> multiply (DVE)
  out[r, e] = sum_dv ctx^T[dv, r] * Wo[dv, e] + bo                       (lhsT=ctx^T, rhs=Wo natural)

All matmul operands bf16 (host-cast); PSUM accumulation fp32; biases fp32; output fp32.
Host pre-transposes query/key/value and pre-tiles Wq/Wk/value so every DMA is contiguous.
"""

import sys

if "/opt/trn_rl_repo" not in sys.path:
    sys.path.insert(0, "/opt/trn_rl_repo")

import numpy as np
import ml_dtypes

import concourse.bass as bass
import concourse.tile as tile
from concourse import bacc
from concourse import mybir
from concourse.bass_utils import run_bass_kernel_spmd

BF16 = ml_dtypes.bfloat16

B, LQ, LK = 8, 512, 1024
EMB, VIN, VOUT = 1024, 512, 1024
H = 16
DH = EMB // H   # 64
DV = VOUT // H  # 64
P = 128
SCALE = 1.0 / 8.0  # 1/sqrt(DH)

EC = EMB // P    # 8 e-chunks
KC = EMB // P    # 8 k-chunks (emb contraction)
VC = VIN // P    # 4 vin-chunks
JC = LK // P     # 8 lk-chunks
MC = LQ // P     # 4 row-chunks
NQ = LQ          # moving free dim for scores/ctx (=512)

F32 = mybir.dt.float32
BF = mybir.dt.bfloat16


def build_module():
    nc = bacc.Bacc("TRN2", target_bir_lowering=False)

    # ---- DRAM parameters (per-core layouts, host pre-arranged) ----
    qT = nc.declare_dram_parameter("qT", [EMB, LQ], BF, isOutput=False)          # query[b].T
    keyT = nc.declare_dram_parameter("keyT", [VIN, LK], BF, isOutput=False)      # key[b].T
    # value[b].T tiled: [j=8][p=128 vin-in-chunk ... ] -> [JC, P, VC, P] (see host prep)
    valT = nc.declare_dram_parameter("valT", [JC, P, VC, P], BF, isOutput=False)
    # Wq column-tiles, SBUF-ready: wq[m, p, c, e] = Wq[c*128+p, m*128+e]
    wq = nc.declare_dram_parameter("wq", [EC, P, KC, P], BF, isOutput=False)
    wk = nc.declare_dram_parameter("wk", [EC, P, VC, P], BF, isOutput=False)
    wv = nc.declare_dram_parameter("wv", [VIN, VOUT], BF, isOutput=False)        # natural
    wo = nc.declare_dram_parameter("wo", [VOUT, EMB], BF, isOutput=False)        # natural
    bq = nc.declare_dram_parameter("bq", [P, EC], F32, isOutput=False)
    bk = nc.declare_dram_parameter("bk", [P, EC], F32, isOutput=False)
    bv = nc.declare_dram_parameter("bv", [VOUT], F32, isOutput=False)
    bo = nc.declare_dram_parameter("bo", [EMB], F32, isOutput=False)
    out = nc.declare_dram_parameter("out", [LQ, EMB], F32, isOutput=True)

    with tile.TileContext(nc) as tc:
        with (
            tc.tile_pool(name="res", bufs=1) as res,          # resident tiles
            tc.tile_pool(name="wstream", bufs=3) as wstream,  # streamed weight tiles
            tc.tile_pool(name="acts", bufs=1) as acts,        # produced activations (resident)
            tc.tile_pool(name="expp", bufs=36) as expp,       # exp score tiles
            tc.tile_pool(name="smalls", bufs=2) as smalls,    # recip / bcast tiles
            tc.tile_pool(name="outp", bufs=2) as outp,
            tc.tile_pool(name="stage", bufs=6) as stage,
            tc.tile_pool(name="dramp", bufs=4, space="DRAM") as dramp,        # output staging
        ):
            from contextlib import ExitStack
            ps_stack = ExitStack()
            psum2 = ps_stack.enter_context(
                tc.tile_pool(name="psum2", bufs=3, space="PSUM"))  # 2-bank tiles
            psum1 = ps_stack.enter_context(
                tc.tile_pool(name="psum1", bufs=2, space="PSUM"))  # 1-bank tiles

            # ---- PE warm-up: dummy matmuls fill the DMA-bound startup window
            # and push the HAM ramp to full clock before real matmuls arrive.
            warm = res.tile([P, NQ], BF, tag="warm")
            nc.vector.memset(warm, 0.0)
            wps = psum1.tile([P, NQ], F32, tag="p1", name="warm_ps")
            for _ in range(36):
                nc.tensor.matmul(wps, lhsT=warm[:, :P], rhs=warm, start=True, stop=True)

            # ---- resident + stream DMAs (per-queue first-need order) ----
            # SP queue: wk0, keyT, bk, wq0, vt0, then later pairs' streams
            wk_t = [wstream.tile([P, VC, P], BF, tag="wk_m", name=f"wk_m{p}") for p in range(EC)]
            wq_t = [wstream.tile([P, KC, P], BF, tag="wq_m", name=f"wq_m{p}") for p in range(EC)]
            vt_t = [wstream.tile([P, VC, P], BF, tag="vt", name=f"vt{p}") for p in range(EC)]
            nc.sync.dma_start(out=wk_t[0], in_=wk[0])
            keyT_in = []
            for c in range(VC):
                t = res.tile([P, LK], BF, tag=f"keyT_{c}", name=f"keyT_{c}")
                nc.sync.dma_start(out=t, in_=keyT[c * P:(c + 1) * P, :])
                keyT_in.append(t)
            bk_sb = res.tile([P, EC], F32, tag="bk")
            nc.sync.dma_start(out=bk_sb, in_=bk[:, :])
            nc.sync.dma_start(out=wq_t[0], in_=wq[0])
            nc.sync.dma_start(out=vt_t[0], in_=valT[0])
            # qT split across SP + POOL queues for parallel load
            qT_in = []
            for c in range(KC):
                t = res.tile([P, LQ], BF, tag=f"qT_{c}", name=f"qT_{c}")
                eng_q = nc.sync if c < 4 else nc.gpsimd
                eng_q.dma_start(out=t, in_=qT[c * P:(c + 1) * P, :])
                qT_in.append(t)
            bq_sb = res.tile([P, EC], F32, tag="bq")
            nc.gpsimd.dma_start(out=bq_sb, in_=bq[:, :])
            wv_in = []
            for c in range(VC):
                t = res.tile([P, VOUT], BF, tag=f"wv_{c}", name=f"wv_{c}")
                nc.gpsimd.dma_start(out=t, in_=wv[c * P:(c + 1) * P, :])
                wv_in.append(t)
            bv_bc = res.tile([P, VOUT], F32, tag="bv")
            nc.gpsimd.dma_start(out=bv_bc, in_=bv[None, :].to_broadcast((P, VOUT)))

            # ---- produced activations (resident SBUF) ----
            qT_sb = [acts.tile([P, LQ], BF, tag=f"qTs_{m}", name=f"qTs_{m}") for m in range(EC)]
            kT_sb = [acts.tile([P, LK], BF, tag=f"kTs_{m}", name=f"kTs_{m}") for m in range(EC)]
            v_ext = [acts.tile([P, H, DV + 1], BF, tag=f"vext_{j}", name=f"vext_{j}") for j in range(JC)]
            ctxT_sb = [acts.tile([P, LQ], BF, tag=f"ctxT_{c}", name=f"ctxT_{c}") for c in range(VOUT // P)]

            exp_sb = {}  # (h, t) -> tile, filled during attention

            def emit_scores(h):
                off = DH * (h % 2)
                pair = h // 2
                for t in range(4):
                    sc = psum2.tile([P, 2, NQ], F32, tag="p2", name=f"sc_{h}_{t}")
                    for i in range(2):
                        j = 2 * t + i
                        nc.tensor.matmul(
                            sc[:, i, :],
                            lhsT=kT_sb[pair][off:off + DH, j * P:(j + 1) * P],
                            rhs=qT_sb[pair][off:off + DH, :],
                            start=True,
                            stop=True,
                        )
                    e = expp.tile([P, 2, NQ], BF, tag="exp", name=f"exp_{h}_{t}")
                    nc.scalar.activation(
                        out=e.rearrange("p a b -> p (a b)"),
                        in_=sc.rearrange("p a b -> p (a b)"),
                        func=mybir.ActivationFunctionType.Exp,
                        scale=SCALE,
                    )
                    exp_sb[(h, t)] = e

            stage_sb = {}
            totg_sb = {}

            def emit_ctx(h):
                off = DH * (h % 2)
                pair = h // 2
                cx = psum1.tile([DV + 1, NQ], F32, tag="p1", name=f"cx_{h}")
                for t in range(4):
                    e = exp_sb[(h, t)]
                    for i in range(2):
                        j = 2 * t + i
                        nc.tensor.matmul(
                            cx,
                            lhsT=v_ext[j][:, h, :],
                            rhs=e[:, i, :],
                            start=(j == 0),
                            stop=(j == JC - 1),
                        )
                # ctx_ext (incl. totals row) -> SBUF staging; frees PSUM fast
                st = stage.tile([DV + 1, NQ], F32, tag="stg", name=f"st_{h}")
                nc.vector.tensor_copy(out=st, in_=cx[:])
                g = h // 4
                if h % 4 == 0:
                    totg_sb[g] = smalls.tile([4, NQ], F32, tag="totg", name=f"tg_{g}")
                nc.sync.dma_start(out=totg_sb[g][h % 4:h % 4 + 1, :],
                                  in_=st[DV:DV + 1, :])
                stage_sb[h] = st

            def emit_norm_group(g):
                h0 = 4 * g
                rec = smalls.tile([4, NQ], F32, tag="recg", name=f"rec_{g}")
                nc.vector.reciprocal(out=rec, in_=totg_sb[g])
                recd = dramp.tile([4, NQ], F32, tag="recd", name=f"recd_{g}")
                nc.sync.dma_start(out=recd, in_=rec)
                for hh in range(h0, h0 + 4):
                    off = DH * (hh % 2)
                    pair = hh // 2
                    bc = smalls.tile([DV, NQ], F32, tag="bcast", name=f"bc_{hh}")
                    nc.sync.dma_start(
                        out=bc, in_=recd[hh - h0:hh - h0 + 1, :].to_broadcast((DV, NQ)))
                    nc.vector.tensor_tensor(
                        out=ctxT_sb[pair][off:off + DV, :],
                        in0=stage_sb[hh][0:DV, :],
                        in1=bc[:],
                        op=mybir.AluOpType.mult,
                    )

            # =========================================================
            # Phase 1 (per pair p): K-proj(p), Q-proj(p), scores(early),
            # V-proj(j=p)
            # =========================================================
            EARLY = {0: [0, 1], 1: [2, 3], 2: [4], 3: [5], 4: [6], 5: [7], 6: [8]}
            for p in range(EC):
                if p > 0:
                    nc.sync.dma_start(out=wk_t[p], in_=wk[p])
                    nc.sync.dma_start(out=wq_t[p], in_=wq[p])
                    nc.sync.dma_start(out=vt_t[p], in_=valT[p])
                # ---- K projection, e-chunk p ----
                wk_m = wk_t[p]
                kq = psum2.tile([P, 2, NQ], F32, tag="p2")
                for n in range(2):
                    for c in range(VC):
                        nc.tensor.matmul(
                            kq[:, n, :],
                            lhsT=wk_m[:, c, :],
                            rhs=keyT_in[c][:, n * NQ:(n + 1) * NQ],
                            start=(c == 0),
                            stop=(c == VC - 1),
                        )
                nc.vector.tensor_tensor(
                    out=kT_sb[p][:], in0=kq.rearrange("p a b -> p (a b)"),
                    in1=bk_sb[:, p:p + 1].to_broadcast((P, LK)),
                    op=mybir.AluOpType.add,
                )

                # ---- Q projection, e-chunk p ----
                wq_m = wq_t[p]
                qp = psum1.tile([P, NQ], F32, tag="p1")
                for c in range(KC):
                    nc.tensor.matmul(
                        qp,
                        lhsT=wq_m[:, c, :],
                        rhs=qT_in[c][:],
                        start=(c == 0),
                        stop=(c == KC - 1),
                    )
                nc.vector.tensor_tensor(
                    out=qT_sb[p][:], in0=qp[:],
                    in1=bq_sb[:, p:p + 1].to_broadcast((P, LQ)),
                    op=mybir.AluOpType.add,
                )

                # ---- early scores (feed ACT ASAP), before V-proj ----
                for eh in EARLY.get(p, []):
                    emit_scores(eh)

                # ---- V projection, lk-chunk j=p ----
                vt = vt_t[p]
                vp = psum2.tile([P, 2, NQ], F32, tag="p2")
                for n in range(2):
                    for c in range(VC):
                        nc.tensor.matmul(
                            vp[:, n, :],
                            lhsT=vt[:, c, :],
                            rhs=wv_in[c][:, n * NQ:(n + 1) * NQ],
                            start=(c == 0),
                            stop=(c == VC - 1),
                        )
                for n in range(2):
                    nc.vector.tensor_tensor(
                        out=v_ext[p][:, n * 8:(n + 1) * 8, 0:DV],
                        in0=vp[:, n, :].rearrange("p (a b) -> p a b", a=8),
                        in1=bv_bc[:, n * NQ:(n + 1) * NQ].rearrange("p (a b) -> p a b", a=8),
                        op=mybir.AluOpType.add,
                    )
                nc.vector.memset(v_ext[p][:, :, DV:DV + 1], 1.0)

            # =========================================================
            # Phase 2 per head: scores -> exp (ACT) -> ctx, with
            # lookahead-1 on scores so ACT stays ahead of PE.
            # =========================================================
            wo_in = []
            for c in range(VOUT // P):
                t = res.tile([P, EMB], BF, tag=f"qT_{c}", name=f"wo_{c}")
                nc.gpsimd.dma_start(out=t, in_=wo[c * P:(c + 1) * P, :])
                wo_in.append(t)
            bo_bc = res.tile([P, EMB], F32, tag="bo")
            nc.gpsimd.dma_start(out=bo_bc, in_=bo[None, :].to_broadcast((P, EMB)))

            LOOK = 8
            for h in range(H):
                if h + LOOK < H and h + LOOK > 8:
                    emit_scores(h + LOOK)
                emit_ctx(h)
                if h % 4 == 3:
                    emit_norm_group(h // 4)

            # =========================================================
            # Phase 3: output projection
            # =========================================================
            ps_stack.close()
            with tc.tile_pool(name="psum_o", bufs=4, space="PSUM") as psum_o:
                pos = [psum_o.tile([P, 2, NQ], F32, tag="po", name=f"po_{m}")
                       for m in range(MC)]
                for c in range(VOUT // P):
                    for m in range(MC):
                        for n in range(2):
                            nc.tensor.matmul(
                                pos[m][:, n, :],
                                lhsT=ctxT_sb[c][:, m * P:(m + 1) * P],
                                rhs=wo_in[c][:, n * NQ:(n + 1) * NQ],
                                start=(c == 0),
                                stop=(c == VOUT // P - 1),
                            )
                for m in range(MC):
                    ot = outp.tile([P, EMB], F32, tag="out", name=f"ot_{m}")
                    nc.vector.tensor_tensor(
                        out=ot,
                        in0=pos[m].rearrange("p a b -> p (a b)"),
                        in1=bo_bc[:],
                        op=mybir.AluOpType.add,
                    )
                    nc.sync.dma_start(out=out[m * P:(m + 1) * P, :], in_=ot)

    nc.compile()
    return nc


_NC_CACHE = {}


def _get_module():
    if "nc" not in _NC_CACHE:
        _NC_CACHE["nc"] = build_module()
    return _NC_CACHE["nc"]


def _prep_core_inputs(query, key, value, wq_t, wk_t, wv_b, wo_b, bq, bk, bv, bo, b):
    qT = np.ascontiguousarray(query[b].T).astype(BF16)              # [1024, 512]
    keyT = np.ascontiguousarray(key[b].T).astype(BF16)              # [512, 1024]
    # value[b].T [512, 1024] -> [j, p(vin%128), c(vin//128), e] tiles
    vT = value[b].T.reshape(VC, P, JC, P).transpose(2, 1, 0, 3)     # [8, 128, 4, 128]
    valT = np.ascontiguousarray(vT).astype(BF16)
    return {
        "qT": qT, "keyT": keyT, "valT": valT,
        "wq": wq_t, "wk": wk_t, "wv": wv_b, "wo": wo_b,
        "bq": bq, "bk": bk, "bv": bv, "bo": bo,
    }


def kernel(query, key, value, Wq, bq, Wk, bk, Wv, bv, Wo, bo, _want_profile=False):
    nc = _get_module()

    # weight prep (shared across cores)
    # wq[m, p, c, e] = Wq[c*128+p, m*128+e]
    wq_t = np.ascontiguousarray(
        Wq.reshape(KC, P, EC, P).transpose(2, 1, 0, 3)).astype(BF16)  # [8, 128, 8, 128]
    wk_t = np.ascontiguousarray(
        Wk.reshape(VC, P, EC, P).transpose(2, 1, 0, 3)).astype(BF16)  # [8, 128, 4, 128]
    wv_b = Wv.astype(BF16)
    wo_b = Wo.astype(BF16)
    bq32 = np.ascontiguousarray(np.asarray(bq, np.float32).reshape(EC, P).T)  # [128, 8]
    bk32 = np.ascontiguousarray(np.asarray(bk, np.float32).reshape(EC, P).T)  # [128, 8]
    bv32 = np.asarray(bv, np.float32)
    bo32 = np.asarray(bo, np.float32)

    in_maps = [
        _prep_core_inputs(query, key, value, wq_t, wk_t, wv_b, wo_b,
                          bq32, bk32, bv32, bo32, b)
        for b in range(B)
    ]
    res = run_bass_kernel_spmd(nc, in_maps, core_ids=list(range(B)),
                               trace=_want_profile)
    out = np.stack([res.results[c]["out"] for c in range(B)], axis=0)
    if _want_profile:
        return out, res
    return out
